# revision 92
# baseline (speedup 1.0000x reference)
"""Trainium2 Bass kernel for nn_AttentionFlow (BiDAF-style attention flow).

Math (per batch b, biases cancel):
  s[t,i]   = <c_t,w_c> + <q_i,w_q> + <c_t*q_i, w_cq>  (+ biases)
  a        = softmax_i(s)          -> c2q = a @ q
  beta     = softmax_t(max_i s)    -> q2c = beta^T c
  out      = [c | c2q | c*c2q | c*q2c]

Design: everything is computed in the TRANSPOSED score domain.
  s'^T[i,t] = qa^T @ c^T + sq (x) 1        (qa[d,i] = q^T*w_cq + w_c)
  e^T = exp(s'^T)   [i on partitions, t free]  -- born as mm2's lhsT,
  c2q[t,d] = (e^T)^T @ q                       -- natural output layout,
  r[t]     = (e^T)^T @ 1                       -- N=1 matmuls, shared weights,
  g[t]     = max_i e^T[i,t] = exp(max_i s')    -- GPSIMD partition all-reduce,
  beta     = g/sum(g), q2c = beta^T c via PE with per-group accumulation.

This removes all 64 E^T PE transposes of the naive layout; the only PE
transposes are c^T (64, bf16), q^T (16, bf16) and 16 skinny g-column flips.
t and i orderings are arbitrary (softmax/contractions are order-invariant,
outputs re-addressed by AP), so row->partition maps are chosen for DMA
contiguity: t = p*16 + j, i = 4*p + k.

Sharding: data-parallel over batch, one batch element per NeuronCore (8).
"""

import numpy as np

N_CORES = 8
T, I, D = 2048, 512, 512
TT = T // 128   # 16 row tiles
KC = 4          # 128-chunks of D
IC = 4          # 128-chunks of I
NG = 4          # t-groups of 512 rows (4 tiles each)

DEFAULT_OPTS = dict(
    bufs_work=3, bufs_out=3, ps_tr_bufs=3, ps_s_bufs=2, ps_mm2_bufs=2,
    ct_acts=10,      # how many of the LAST ct copies go on ACT (rest DVE)
    o2_acts=4,       # how many of the 16 o2 scales go on ACT (rest DVE)
    o3_dve=14,       # how many of the 16 o3 muls go on DVE (rest GPSIMD)
    o4_dve=12,       # how many of the 16 o4 muls go on DVE (rest GPSIMD)
    skip_out=False,
)

_BUILT = None


def _build(reps=1, timing_mode=False, opts=None):
    import concourse.tile as tile
    from concourse import bacc, bass_isa, mybir
    from concourse.masks import make_identity

    o = dict(DEFAULT_OPTS)
    if opts:
        o.update(opts)

    f32 = mybir.dt.float32
    bf16 = mybir.dt.bfloat16
    AF = mybir.ActivationFunctionType
    AX = mybir.AxisListType
    ALU = mybir.AluOpType

    nc = bacc.Bacc("TRN2", target_bir_lowering=False, debug=False,
                   num_devices=N_CORES)
    c_d = nc.dram_tensor("c", [T, D], bf16, kind="ExternalInput").ap()
    q_d = nc.dram_tensor("q", [I, D], bf16, kind="ExternalInput").ap()
    wc_d = nc.dram_tensor("wc", [D], f32, kind="ExternalInput").ap()
    wq_d = nc.dram_tensor("wq", [D], f32, kind="ExternalInput").ap()
    wcq_d = nc.dram_tensor("wcq", [D], f32, kind="ExternalInput").ap()
    out_kind = "Internal" if timing_mode else "ExternalOutput"
    out_d = nc.dram_tensor("out", [T, 4 * D], f32, kind=out_kind).ap()
    tick_d = (nc.dram_tensor("tick", [1, 1], f32, kind="ExternalOutput").ap()
              if timing_mode else None)

    with tile.TileContext(nc) as tc:
        with (
            tc.tile_pool(name="const", bufs=1) as constp,
            tc.tile_pool(name="big", bufs=1) as bigp,
            tc.tile_pool(name="work", bufs=o["bufs_work"]) as workp,
            tc.tile_pool(name="outp", bufs=o["bufs_out"]) as outp,
            tc.tile_pool(name="ps_tr", bufs=o["ps_tr_bufs"],
                         space="PSUM") as ps_tr,
            tc.tile_pool(name="ps_s", bufs=o["ps_s_bufs"],
                         space="PSUM") as ps_s,
            tc.tile_pool(name="ps_mm2", bufs=o["ps_mm2_bufs"],
                         space="PSUM") as ps_mm2,
            tc.tile_pool(name="ps_q2c", bufs=1, space="PSUM") as ps_q2c,
        ):
            for _rep in range(reps):
                crs = c_d.rearrange("(p j) d -> p j d", j=TT)
                ors = out_d.rearrange("(p j) w -> p j w", j=TT)
                qrs = q_d.rearrange("(p k) d -> p k d", k=IC)

                # ---------------- input DMAs (head-latency ordered) --------
                # q d-chunk 0 first (unblocks q^T), then c group 0, then the
                # rest of q; weights ride the scalar queue.
                q_sb = bigp.tile([128, IC, D], bf16, tag="q_sb")
                c_gb = [bigp.tile([128, 4, D], bf16, tag=f"cg{g}",
                                  name=f"cg{g}") for g in range(NG)]
                nc.sync.dma_start(q_sb[:, :, 0:256], qrs[:, :, 0:256])
                nc.sync.dma_start(q_sb[:, :, 256:512], qrs[:, :, 256:512])
                nc.sync.dma_start(c_gb[0][:], crs[:, 0:4, :])

                wcq_col = constp.tile([128, KC], f32, tag="wcq_col")
                nc.scalar.dma_start(wcq_col[:],
                                    wcq_d.rearrange("(a b) -> b a", b=128))
                wc_col = constp.tile([128, KC], f32, tag="wc_col")
                nc.scalar.dma_start(wc_col[:],
                                    wc_d.rearrange("(a b) -> b a", b=128))
                wq_col = constp.tile([128, KC], f32, tag="wq_col")
                nc.scalar.dma_start(wq_col[:],
                                    wq_d.rearrange("(a b) -> b a", b=128))

                # ---------------- constants --------------------------------
                ident_b = constp.tile([128, 128], bf16, tag="idb")
                make_identity(nc, ident_b[:])
                ones_row_b = constp.tile([1, 128], bf16, tag="ones_row_b")
                nc.vector.memset(ones_row_b[:], 1.0)
                ones_col_b = constp.tile([128, 1], bf16, tag="ones_col_b")
                nc.vector.memset(ones_col_b[:], 1.0)

                # ---------------- q path (bf16) ----------------------------
                # i-map: partition p, chunk k -> i = 4*p + k
                q_bf = q_sb
                qt = bigp.tile([128, KC, I], bf16, tag="qt")
                qa = bigp.tile([128, KC, I], bf16, tag="qa")
                wq_b = constp.tile([128, KC], bf16, tag="wq_b")
                sq_col = constp.tile([128, IC], f32, tag="sq_col")

                def q_path():
                    # q^T (ii = 128*ik + p), qa = q^T*wcq + wc
                    for k in range(KC):
                        pt = ps_tr.tile([128, I], bf16, tag="ps_tr")
                        for ik in range(IC):
                            nc.tensor.transpose(
                                pt[:, ik * 128:(ik + 1) * 128],
                                q_bf[:, ik, k * 128:(k + 1) * 128],
                                ident_b[:])
                        nc.scalar.copy(qt[:, k], pt[:])
                        nc.vector.tensor_scalar(
                            qa[:, k], pt[:], wcq_col[:, k:k + 1],
                            wc_col[:, k:k + 1], op0=ALU.mult, op1=ALU.add)
                def sq_block():
                    # sq as columns [i-part, chunk]: folded into exp's
                    # bias, so mm1 needs no broadcast matmuls. Emitted
                    # after group 0's c^T so PE never waits on qt copies.
                    nc.vector.tensor_copy(wq_b[:], wq_col[:])
                    ps_sq = ps_q2c.tile([128, IC], f32, tag="ps_q2c")
                    for m in range(IC):
                        for k in range(KC):
                            nc.tensor.matmul(ps_sq[:, m:m + 1],
                                             qt[:, k, m * 128:(m + 1) * 128],
                                             wq_b[:, k:k + 1],
                                             start=(k == 0),
                                             stop=(k == KC - 1),
                                             skip_group_check=True)
                    nc.vector.tensor_copy(sq_col[:], ps_sq[:])

                # ---------------- main tiles -------------------------------
                ct_g = [bigp.tile([128, KC, 512], bf16, tag=f"ct{g}",
                                  name=f"ct{g}") for g in range(NG)]
                et_g = [bigp.tile([128, IC, 512], bf16, tag=f"et{g}",
                                  name=f"et{g}") for g in range(NG)]
                rinv_g = [bigp.tile([128, NG], f32, tag=f"rinv{g}",
                                    name=f"rinv{g}") for g in range(NG)]
                gm_g = [bigp.tile([128, 512], bf16, tag=f"gm{g}",
                                  name=f"gm{g}") for g in range(NG)]
                mcol_g = [bigp.tile([128, 4], bf16, tag=f"mc{g}",
                                    name=f"mc{g}") for g in range(NG)]
                o23_g = [outp.tile([128, 4, 1024], f32, tag="o23",
                                   name=f"o23_{g}") for g in range(NG)]
                o4_g = [outp.tile([128, 4, 512], f32, tag="o4",
                                  name=f"o4_{g}") for g in range(NG)]
                oc_g = [outp.tile([128, 4, 512], f32, tag="oc",
                                  name=f"oc_{g}") for g in range(NG)]
                zacc = constp.tile([128, 1], f32, tag="zacc")
                psq2c = [None]

                def c_fine(j):
                    g, b = divmod(j, 4)
                    return c_gb[g][:, b, :]

                _n = dict(ct=0, o2=0, o3=0, o4=0, odma=0)

                def out_dma(dst, src):
                    if o["skip_out"]:
                        return
                    _n["odma"] += 1
                    eng = nc.scalar if _n["odma"] % 2 else nc.sync
                    eng.dma_start(dst, src)

                # ---------------- pipeline stages --------------------------
                def phase1(g):
                    """loads + c^T + mm1 + exp + g-max + q2c partials."""
                    if g + 1 < NG:
                        nc.sync.dma_start(c_gb[g + 1][:],
                                          crs[:, 4 * (g + 1):4 * (g + 2), :])
                    # widen the o1 echo early: GPSIMD is idle here, so the
                    # echo DMA later never waits on the widening copy.
                    nc.gpsimd.tensor_copy(oc_g[g][:], c_gb[g][:])

                    # c^T for this group: ct_g[g][dk, k, 128*b + pc]
                    for k in range(KC):
                        pt = ps_tr.tile([128, 512], bf16, tag="ps_tr")
                        for b in range(4):
                            nc.tensor.transpose(
                                pt[:, b * 128:(b + 1) * 128],
                                c_gb[g][:, b, k * 128:(k + 1) * 128],
                                ident_b[:])
                        _n["ct"] += 1
                        if _n["ct"] > 16 - o["ct_acts"]:
                            nc.scalar.copy(ct_g[g][:, k, :], pt[:])
                        else:
                            nc.vector.tensor_copy(ct_g[g][:, k, :], pt[:])

                    # mm1: s'^T[im, t] = sum_k qa[k,im]^T @ ct; sq[i] rides
                    # exp's per-partition bias.
                    for m in range(IC):
                        ps = ps_s.tile([128, 512], f32, tag="ps_s")
                        for k in range(KC):
                            nc.tensor.matmul(
                                ps[:], qa[:, k, m * 128:(m + 1) * 128],
                                ct_g[g][:, k, :],
                                start=(k == 0), stop=(k == KC - 1),
                                skip_group_check=True)
                        nc.scalar.activation(et_g[g][:, m, :], ps[:], AF.Exp,
                                             bias=sq_col[:, m:m + 1])

                    # g-row: gmax over i = chunk-max (DVE) + partition
                    # all-reduce max (GPSIMD daisy chain); gm rows identical
                    # across partitions.
                    tr0 = workp.tile([128, 512], bf16, tag="tr0")
                    tr1 = workp.tile([128, 512], bf16, tag="tr1")
                    nc.vector.tensor_tensor(tr0[:], et_g[g][:, 0, :],
                                            et_g[g][:, 1, :], op=ALU.max)
                    nc.vector.tensor_tensor(tr1[:], et_g[g][:, 2, :],
                                            et_g[g][:, 3, :], op=ALU.max)
                    nc.vector.tensor_tensor(tr0[:], tr0[:], tr1[:],
                                            op=ALU.max)
                    nc.gpsimd.partition_all_reduce(
                        gm_g[g][:], tr0[:], 128, bass_isa.ReduceOp.max)

                def q2c_partials(g):
                    """Deferred one stage so the PE queue never waits on
                    group g's partition all-reduce: g columns via skinny
                    transposes, then accumulate beta-weighted c and Z."""
                    # [128, 4, 2] so each bf16 column sits 4-byte aligned
                    pmc = ps_tr.tile([128, 4, 2], bf16, tag="ps_tr")
                    for b in range(4):
                        nc.tensor.transpose(
                            pmc[:, b, 0:1],
                            gm_g[g][0:1, b * 128:(b + 1) * 128],
                            ident_b[0:1, 0:1])
                    nc.vector.tensor_copy(mcol_g[g][:], pmc[:, :, 0])
                    if g == 0:
                        psq2c[0] = ps_q2c.tile([1, D], f32, tag="ps_q2c",
                                               name="psq2c")
                    for b in range(4):
                        nc.tensor.matmul(psq2c[0][:], mcol_g[g][:, b:b + 1],
                                         c_gb[g][:, b, :],
                                         start=(g == 0 and b == 0),
                                         stop=(g == NG - 1 and b == 3),
                                         skip_group_check=True)
                    # Z partial: every partition of gm_g holds the full
                    # g-row, so a free-dim sum gives the group Z everywhere.
                    zc = workp.tile([128, 1], f32, tag="zc")
                    nc.vector.reduce_sum(zc[:], gm_g[g][:], axis=AX.X)
                    if g == 0:
                        nc.vector.tensor_copy(zacc[:], zc[:])
                    else:
                        nc.vector.tensor_add(zacc[:], zacc[:], zc[:])

                def mm2_block(g, dma_fine=False):
                    """mm2 + row sums + o2/o3 + output DMA for group g."""
                    pcs = []
                    rps = None
                    for b in range(4):
                        if b % 2 == 0:
                            rps = ps_s.tile([128, 2], f32, tag="ps_s",
                                            name="rps")
                        pc = ps_mm2.tile([128, 512], f32, tag="ps_mm2")
                        pcs.append(pc)
                        for m in range(IC):
                            lhs = et_g[g][:, m, b * 128:(b + 1) * 128]
                            nc.tensor.matmul(pc[:], lhs, q_bf[:, m, :],
                                             start=(m == 0), stop=(m == IC - 1),
                                             skip_group_check=True)
                            nc.tensor.matmul(rps[:, b % 2:b % 2 + 1], lhs,
                                             ones_col_b[:],
                                             start=(m == 0), stop=(m == IC - 1),
                                             skip_group_check=True)
                        if b % 2 == 1:
                            nc.vector.reciprocal(rinv_g[g][:, b - 1:b + 1],
                                                 rps[:])
                    o_t = o23_g[g]
                    for b in range(4):
                        j = 4 * g + b
                        pc = pcs[b]
                        _n["o2"] += 1
                        if _n["o2"] <= o["o2_acts"]:
                            nc.scalar.mul(o_t[:, b, 0:512], pc[:],
                                          rinv_g[g][:, b:b + 1])
                        else:
                            nc.vector.tensor_scalar_mul(o_t[:, b, 0:512],
                                                        pc[:],
                                                        rinv_g[g][:, b:b + 1])
                        _n["o3"] += 1
                        o3e = (nc.vector if _n["o3"] <= o["o3_dve"]
                               else nc.gpsimd)
                        o3e.tensor_mul(o_t[:, b, 512:1024], c_fine(j),
                                       o_t[:, b, 0:512])
                        if dma_fine:
                            out_dma(ors[:, j, 512:1536], o_t[:, b, :])
                    if not dma_fine:
                        out_dma(ors[:, 4 * g:4 * g + 4, 512:1536], o_t[:])
                    # o1 echo DMA behind this group's output (pipe filler)
                    out_dma(ors[:, 4 * g:4 * g + 4, 0:512], oc_g[g][:])

                def o4_block(gs, q2c_bc, dma_fine=False):
                    for g in gs:
                        for b in range(4):
                            j = 4 * g + b
                            _n["o4"] += 1
                            o4e = (nc.vector if _n["o4"] <= o["o4_dve"]
                                   else nc.gpsimd)
                            o4e.tensor_mul(o4_g[g][:, b, :], c_fine(j),
                                           q2c_bc[:])
                            if dma_fine:
                                out_dma(ors[:, j, 1536:2048],
                                        o4_g[g][:, b, :])
                            elif b % 2 == 1:
                                out_dma(
                                    ors[:, 4 * g + b - 1:4 * g + b + 1,
                                        1536:2048],
                                    o4_g[g][:, b - 1:b + 1, :])

                def q2c_finalize():
                    zinv = constp.tile([128, 1], f32, tag="zinv")
                    nc.vector.reciprocal(zinv[:], zacc[:])
                    q2c_u = constp.tile([1, D], bf16, tag="q2c_u")
                    nc.vector.tensor_copy(q2c_u[:], psq2c[0][:])
                    psbc = ps_q2c.tile([128, D], f32, tag="ps_q2c")
                    nc.tensor.matmul(psbc[:], ones_row_b[:], q2c_u[:],
                                     start=True, stop=True,
                                     skip_group_check=True)
                    q2c_bc = constp.tile([128, D], bf16, tag="q2c_bc")
                    nc.scalar.mul(q2c_bc[:], psbc[:], zinv[:])
                    return q2c_bc

                # ---------------- pipelined emission -----------------------
                # mm2 one group behind phase1 so o2/o3 bytes flow early;
                # o4 streams as soon as the beta reduction closes.
                q_path()
                sq_block()
                phase1(0)
                mm2_block(0)
                phase1(1)
                q2c_partials(0)
                mm2_block(1)
                phase1(2)
                q2c_partials(1)
                mm2_block(2)
                phase1(3)
                q2c_partials(2)
                mm2_block(3, dma_fine=True)
                q2c_partials(3)
                q2c_bc = q2c_finalize()
                o4_block([0, 1, 2], q2c_bc)
                o4_block([3], q2c_bc, dma_fine=True)

        if timing_mode:
            with tc.tile_pool(name="tickp", bufs=1) as tickp:
                tk = tickp.tile([1, 1], f32, tag="tick")
                nc.vector.memset(tk[:], 1.0)
                nc.sync.dma_start(tick_d[:], tk[:])

    nc.compile()
    return nc


def _get_built():
    global _BUILT
    if _BUILT is None:
        _BUILT = _build()
    return _BUILT


def kernel(c, q, w_c, b_c, w_q, b_q, w_cq, b_cq):
    """Full inputs in, full output out. Data-parallel over batch on 8 cores.

    Biases cancel mathematically (softmax shift invariance), so b_* are
    accepted but unused.
    """
    from concourse import bass_utils

    nc = _get_built()
    c = np.ascontiguousarray(np.asarray(c, dtype=np.float32))
    q = np.ascontiguousarray(np.asarray(q, dtype=np.float32))
    wc = np.ascontiguousarray(np.asarray(w_c, dtype=np.float32))
    wq = np.ascontiguousarray(np.asarray(w_q, dtype=np.float32))
    wcq = np.ascontiguousarray(np.asarray(w_cq, dtype=np.float32))

    in_maps = [
        {"c": c[b], "q": q[b], "wc": wc, "wq": wq, "wcq": wcq}
        for b in range(N_CORES)
    ]
    res = bass_utils.run_bass_kernel_spmd(
        nc, in_maps, core_ids=list(range(N_CORES)))
    return np.stack([res.results[b]["out"] for b in range(N_CORES)])


# revision 98
# speedup vs baseline: 1.1731x; 1.1731x over previous
"""Trainium2 Bass kernel for nn_AttentionFlow (BiDAF-style attention flow).

Math (per batch b, biases cancel):
  s[t,i]   = <c_t,w_c> + <q_i,w_q> + <c_t*q_i, w_cq>  (+ biases)
  a        = softmax_i(s)          -> c2q = a @ q
  beta     = softmax_t(max_i s)    -> q2c = beta^T c
  out      = [c | c2q | c*c2q | c*q2c]

Design: everything is computed in the TRANSPOSED score domain.
  s'^T[i,t] = qa^T @ c^T + sq (x) 1        (qa[d,i] = q^T*w_cq + w_c)
  e^T = exp(s'^T)   [i on partitions, t free]  -- born as mm2's lhsT,
  c2q[t,d] = (e^T)^T @ q                       -- natural output layout,
  r[t]     = (e^T)^T @ 1                       -- N=1 matmuls, shared weights,
  g[t]     = max_i e^T[i,t] = exp(max_i s')    -- GPSIMD partition all-reduce,
  beta     = g/sum(g), q2c = beta^T c via PE with per-group accumulation.

This removes all 64 E^T PE transposes of the naive layout; the only PE
transposes are c^T (64, bf16), q^T (16, bf16) and 16 skinny g-column flips.
t and i orderings are arbitrary (softmax/contractions are order-invariant,
outputs re-addressed by AP), so row->partition maps are chosen for DMA
contiguity: t = p*16 + j, i = 4*p + k.

Sharding: data-parallel over batch, one batch element per NeuronCore (8).
"""

import numpy as np

N_CORES = 8
T, I, D = 2048, 512, 512
TT = T // 128   # 16 row tiles
KC = 4          # 128-chunks of D
IC = 4          # 128-chunks of I
NG = 4          # t-groups of 512 rows (4 tiles each)

DEFAULT_OPTS = dict(
    bufs_work=3, bufs_out=3, ps_tr_bufs=3, ps_s_bufs=2, ps_mm2_bufs=2,
    ct_acts=10,      # how many of the LAST ct copies go on ACT (rest DVE)
    o2_acts=4,       # how many of the 16 o2 scales go on ACT (rest DVE)
    o3_dve=14,       # how many of the 16 o3 muls go on DVE (rest GPSIMD)
    o4_dve=12,       # how many of the 16 o4 muls go on DVE (rest GPSIMD)
    skip_out=False,
)

_BUILT = None


def _build(reps=1, timing_mode=False, opts=None):
    import concourse.tile as tile
    from concourse import bacc, bass_isa, mybir
    from concourse.masks import make_identity

    o = dict(DEFAULT_OPTS)
    if opts:
        o.update(opts)

    f32 = mybir.dt.float32
    bf16 = mybir.dt.bfloat16
    AF = mybir.ActivationFunctionType
    AX = mybir.AxisListType
    ALU = mybir.AluOpType

    nc = bacc.Bacc("TRN2", target_bir_lowering=False, debug=False,
                   num_devices=N_CORES)
    c_d = nc.dram_tensor("c", [T, D], bf16, kind="ExternalInput").ap()
    q_d = nc.dram_tensor("q", [I, D], bf16, kind="ExternalInput").ap()
    wc_d = nc.dram_tensor("wc", [D], f32, kind="ExternalInput").ap()
    wq_d = nc.dram_tensor("wq", [D], f32, kind="ExternalInput").ap()
    wcq_d = nc.dram_tensor("wcq", [D], f32, kind="ExternalInput").ap()
    out_kind = "Internal" if timing_mode else "ExternalOutput"
    out_d = nc.dram_tensor("out", [T, 4 * D], f32, kind=out_kind).ap()
    tick_d = (nc.dram_tensor("tick", [1, 1], f32, kind="ExternalOutput").ap()
              if timing_mode else None)

    with tile.TileContext(nc) as tc:
        with (
            tc.tile_pool(name="const", bufs=1) as constp,
            tc.tile_pool(name="big", bufs=1) as bigp,
            tc.tile_pool(name="work", bufs=o["bufs_work"]) as workp,
            tc.tile_pool(name="outp", bufs=o["bufs_out"]) as outp,
            tc.tile_pool(name="ps_tr", bufs=o["ps_tr_bufs"],
                         space="PSUM") as ps_tr,
            tc.tile_pool(name="ps_s", bufs=o["ps_s_bufs"],
                         space="PSUM") as ps_s,
            tc.tile_pool(name="ps_mm2", bufs=o["ps_mm2_bufs"],
                         space="PSUM") as ps_mm2,
            tc.tile_pool(name="ps_q2c", bufs=1, space="PSUM") as ps_q2c,
        ):
            for _rep in range(reps):
                crs = c_d.rearrange("(p j) d -> p j d", j=TT)
                ors = out_d.rearrange("(p j) w -> p j w", j=TT)
                qrs = q_d.rearrange("(p k) d -> p k d", k=IC)

                # ---------------- input DMAs (head-latency ordered) --------
                # q d-chunk 0 first (unblocks q^T), then c group 0, then the
                # rest of q; weights ride the scalar queue.
                q_sb = bigp.tile([128, IC, D], bf16, tag="q_sb")
                c_gb = [bigp.tile([128, 4, D], bf16, tag=f"cg{g}",
                                  name=f"cg{g}") for g in range(NG)]
                nc.sync.dma_start(q_sb[:, :, 0:256], qrs[:, :, 0:256])
                nc.sync.dma_start(q_sb[:, :, 256:512], qrs[:, :, 256:512])
                nc.sync.dma_start(c_gb[0][:], crs[:, 0:4, :])

                wcq_col = constp.tile([128, KC], f32, tag="wcq_col")
                nc.scalar.dma_start(wcq_col[:],
                                    wcq_d.rearrange("(a b) -> b a", b=128))
                wc_col = constp.tile([128, KC], f32, tag="wc_col")
                nc.scalar.dma_start(wc_col[:],
                                    wc_d.rearrange("(a b) -> b a", b=128))
                wq_col = constp.tile([128, KC], f32, tag="wq_col")
                nc.scalar.dma_start(wq_col[:],
                                    wq_d.rearrange("(a b) -> b a", b=128))

                # ---------------- constants --------------------------------
                ident_b = constp.tile([128, 128], bf16, tag="idb")
                make_identity(nc, ident_b[:])
                ones_row_b = constp.tile([1, 128], bf16, tag="ones_row_b")
                nc.vector.memset(ones_row_b[:], 1.0)
                ones_col_b = constp.tile([128, 1], bf16, tag="ones_col_b")
                nc.vector.memset(ones_col_b[:], 1.0)

                # ---------------- q path (bf16) ----------------------------
                # i-map: partition p, chunk k -> i = 4*p + k
                q_bf = q_sb
                qt = bigp.tile([128, KC, I], bf16, tag="qt")
                qa = bigp.tile([128, KC, I], bf16, tag="qa")
                wq_b = constp.tile([128, KC], bf16, tag="wq_b")
                sq_col = constp.tile([128, IC], f32, tag="sq_col")

                def q_path():
                    # q^T (ii = 128*ik + p), qa = q^T*wcq + wc
                    for k in range(KC):
                        pt = ps_tr.tile([128, I], bf16, tag="ps_tr")
                        for ik in range(IC):
                            nc.tensor.transpose(
                                pt[:, ik * 128:(ik + 1) * 128],
                                q_bf[:, ik, k * 128:(k + 1) * 128],
                                ident_b[:])
                        nc.scalar.copy(qt[:, k], pt[:])
                        nc.vector.tensor_scalar(
                            qa[:, k], pt[:], wcq_col[:, k:k + 1],
                            wc_col[:, k:k + 1], op0=ALU.mult, op1=ALU.add)
                def sq_block():
                    # sq as columns [i-part, chunk]: folded into exp's
                    # bias, so mm1 needs no broadcast matmuls. Emitted
                    # after group 0's c^T so PE never waits on qt copies.
                    nc.vector.tensor_copy(wq_b[:], wq_col[:])
                    ps_sq = ps_q2c.tile([128, IC], f32, tag="ps_q2c")
                    for m in range(IC):
                        for k in range(KC):
                            nc.tensor.matmul(ps_sq[:, m:m + 1],
                                             qt[:, k, m * 128:(m + 1) * 128],
                                             wq_b[:, k:k + 1],
                                             start=(k == 0),
                                             stop=(k == KC - 1),
                                             skip_group_check=True)
                    nc.vector.tensor_copy(sq_col[:], ps_sq[:])

                # ---------------- main tiles -------------------------------
                ct_g = [bigp.tile([128, KC, 512], bf16, tag=f"ct{g}",
                                  name=f"ct{g}") for g in range(NG)]
                et_g = [bigp.tile([128, IC, 512], bf16, tag=f"et{g}",
                                  name=f"et{g}") for g in range(NG)]
                rinv_g = [bigp.tile([128, NG], f32, tag=f"rinv{g}",
                                    name=f"rinv{g}") for g in range(NG)]
                gm_g = [bigp.tile([128, 512], bf16, tag=f"gm{g}",
                                  name=f"gm{g}") for g in range(NG)]
                mcol_g = [bigp.tile([128, 4], bf16, tag=f"mc{g}",
                                    name=f"mc{g}") for g in range(NG)]
                o23_g = [outp.tile([128, 4, 1024], f32, tag="o23",
                                   name=f"o23_{g}") for g in range(NG)]
                o4_g = [outp.tile([128, 4, 512], f32, tag="o4",
                                  name=f"o4_{g}") for g in range(NG)]
                oc_g = [outp.tile([128, 4, 512], f32, tag="oc",
                                  name=f"oc_{g}") for g in range(NG)]
                zacc = constp.tile([128, 1], f32, tag="zacc")
                psq2c = [None]

                def c_fine(j):
                    g, b = divmod(j, 4)
                    return c_gb[g][:, b, :]

                _n = dict(ct=0, o2=0, o3=0, o4=0, odma=0)

                def out_dma(dst, src):
                    if o["skip_out"]:
                        return
                    _n["odma"] += 1
                    eng = nc.scalar if _n["odma"] % 2 else nc.sync
                    eng.dma_start(dst, src)

                # ---------------- pipeline stages --------------------------
                def phase1(g):
                    """loads + c^T + mm1 + exp + g-max + q2c partials."""
                    if g + 1 < NG:
                        nc.sync.dma_start(c_gb[g + 1][:],
                                          crs[:, 4 * (g + 1):4 * (g + 2), :])
                    # widen the o1 echo early: GPSIMD is idle here, so the
                    # echo DMA later never waits on the widening copy.
                    nc.gpsimd.tensor_copy(oc_g[g][:], c_gb[g][:])

                    # c^T for this group: ct_g[g][dk, k, 128*b + pc]
                    for k in range(KC):
                        pt = ps_tr.tile([128, 512], bf16, tag="ps_tr")
                        for b in range(4):
                            nc.tensor.transpose(
                                pt[:, b * 128:(b + 1) * 128],
                                c_gb[g][:, b, k * 128:(k + 1) * 128],
                                ident_b[:])
                        _n["ct"] += 1
                        if _n["ct"] > 16 - o["ct_acts"]:
                            nc.scalar.copy(ct_g[g][:, k, :], pt[:])
                        else:
                            nc.vector.tensor_copy(ct_g[g][:, k, :], pt[:])

                    # mm1: s'^T[im, t] = sum_k qa[k,im]^T @ ct; sq[i] rides
                    # exp's per-partition bias.
                    for m in range(IC):
                        ps = ps_s.tile([128, 512], f32, tag="ps_s")
                        for k in range(KC):
                            nc.tensor.matmul(
                                ps[:], qa[:, k, m * 128:(m + 1) * 128],
                                ct_g[g][:, k, :],
                                start=(k == 0), stop=(k == KC - 1),
                                skip_group_check=True)
                        nc.scalar.activation(et_g[g][:, m, :], ps[:], AF.Exp,
                                             bias=sq_col[:, m:m + 1])

                    # g-row: gmax over i = chunk-max (DVE) + partition
                    # all-reduce max (GPSIMD daisy chain); gm rows identical
                    # across partitions.
                    tr0 = workp.tile([128, 512], bf16, tag="tr0")
                    tr1 = workp.tile([128, 512], bf16, tag="tr1")
                    nc.vector.tensor_tensor(tr0[:], et_g[g][:, 0, :],
                                            et_g[g][:, 1, :], op=ALU.max)
                    nc.vector.tensor_tensor(tr1[:], et_g[g][:, 2, :],
                                            et_g[g][:, 3, :], op=ALU.max)
                    nc.vector.tensor_tensor(tr0[:], tr0[:], tr1[:],
                                            op=ALU.max)
                    nc.gpsimd.partition_all_reduce(
                        gm_g[g][:], tr0[:], 128, bass_isa.ReduceOp.max)

                def q2c_partials(g):
                    """Deferred one stage so the PE queue never waits on
                    group g's partition all-reduce: g columns via skinny
                    transposes, then accumulate beta-weighted c and Z."""
                    # [128, 4, 2] so each bf16 column sits 4-byte aligned
                    pmc = ps_tr.tile([128, 4, 2], bf16, tag="ps_tr")
                    for b in range(4):
                        nc.tensor.transpose(
                            pmc[:, b, 0:1],
                            gm_g[g][0:1, b * 128:(b + 1) * 128],
                            ident_b[0:1, 0:1])
                    nc.vector.tensor_copy(mcol_g[g][:], pmc[:, :, 0])
                    if g == 0:
                        psq2c[0] = ps_q2c.tile([1, D], f32, tag="ps_q2c",
                                               name="psq2c")
                    for b in range(4):
                        nc.tensor.matmul(psq2c[0][:], mcol_g[g][:, b:b + 1],
                                         c_gb[g][:, b, :],
                                         start=(g == 0 and b == 0),
                                         stop=(g == NG - 1 and b == 3),
                                         skip_group_check=True)
                    # Z partial: every partition of gm_g holds the full
                    # g-row, so a free-dim sum gives the group Z everywhere.
                    zc = workp.tile([128, 1], f32, tag="zc")
                    nc.vector.reduce_sum(zc[:], gm_g[g][:], axis=AX.X)
                    if g == 0:
                        nc.vector.tensor_copy(zacc[:], zc[:])
                    else:
                        nc.vector.tensor_add(zacc[:], zacc[:], zc[:])

                def mm2_block(g, dma_fine=False):
                    """mm2 + row sums + o2/o3 + output DMA for group g."""
                    pcs = []
                    rps = None
                    for b in range(4):
                        if b % 2 == 0:
                            rps = ps_s.tile([128, 2], f32, tag="ps_s",
                                            name="rps")
                        pc = ps_mm2.tile([128, 512], f32, tag="ps_mm2")
                        pcs.append(pc)
                        for m in range(IC):
                            lhs = et_g[g][:, m, b * 128:(b + 1) * 128]
                            nc.tensor.matmul(pc[:], lhs, q_bf[:, m, :],
                                             start=(m == 0), stop=(m == IC - 1),
                                             skip_group_check=True)
                            nc.tensor.matmul(rps[:, b % 2:b % 2 + 1], lhs,
                                             ones_col_b[:],
                                             start=(m == 0), stop=(m == IC - 1),
                                             skip_group_check=True)
                        if b % 2 == 1:
                            nc.vector.reciprocal(rinv_g[g][:, b - 1:b + 1],
                                                 rps[:])
                    o_t = o23_g[g]
                    for b in range(4):
                        j = 4 * g + b
                        pc = pcs[b]
                        _n["o2"] += 1
                        if _n["o2"] <= o["o2_acts"]:
                            nc.scalar.mul(o_t[:, b, 0:512], pc[:],
                                          rinv_g[g][:, b:b + 1])
                        else:
                            nc.vector.tensor_scalar_mul(o_t[:, b, 0:512],
                                                        pc[:],
                                                        rinv_g[g][:, b:b + 1])
                        _n["o3"] += 1
                        o3e = (nc.vector if _n["o3"] <= o["o3_dve"]
                               else nc.gpsimd)
                        o3e.tensor_mul(o_t[:, b, 512:1024], c_fine(j),
                                       o_t[:, b, 0:512])
                        if dma_fine:
                            out_dma(ors[:, j, 512:1536], o_t[:, b, :])
                    if not dma_fine:
                        out_dma(ors[:, 4 * g:4 * g + 4, 512:1536], o_t[:])
                    # o1 echo DMA behind this group's output (pipe filler)
                    out_dma(ors[:, 4 * g:4 * g + 4, 0:512], oc_g[g][:])

                def o4_block(gs, q2c_bc, dma_fine=False):
                    for g in gs:
                        for b in range(4):
                            j = 4 * g + b
                            _n["o4"] += 1
                            o4e = (nc.vector if _n["o4"] <= o["o4_dve"]
                                   else nc.gpsimd)
                            o4e.tensor_mul(o4_g[g][:, b, :], c_fine(j),
                                           q2c_bc[:])
                            if dma_fine:
                                out_dma(ors[:, j, 1536:2048],
                                        o4_g[g][:, b, :])
                            elif b % 2 == 1:
                                out_dma(
                                    ors[:, 4 * g + b - 1:4 * g + b + 1,
                                        1536:2048],
                                    o4_g[g][:, b - 1:b + 1, :])

                def q2c_finalize():
                    zinv = constp.tile([128, 1], f32, tag="zinv")
                    nc.vector.reciprocal(zinv[:], zacc[:])
                    q2c_u = constp.tile([1, D], bf16, tag="q2c_u")
                    nc.vector.tensor_copy(q2c_u[:], psq2c[0][:])
                    psbc = ps_q2c.tile([128, D], f32, tag="ps_q2c")
                    nc.tensor.matmul(psbc[:], ones_row_b[:], q2c_u[:],
                                     start=True, stop=True,
                                     skip_group_check=True)
                    q2c_bc = constp.tile([128, D], bf16, tag="q2c_bc")
                    nc.scalar.mul(q2c_bc[:], psbc[:], zinv[:])
                    return q2c_bc

                # ---------------- pipelined emission -----------------------
                # mm2 one group behind phase1 so o2/o3 bytes flow early;
                # o4 streams as soon as the beta reduction closes.
                q_path()
                sq_block()
                phase1(0)
                mm2_block(0)
                phase1(1)
                q2c_partials(0)
                mm2_block(1)
                phase1(2)
                q2c_partials(1)
                mm2_block(2)
                phase1(3)
                q2c_partials(2)
                mm2_block(3, dma_fine=True)
                q2c_partials(3)
                q2c_bc = q2c_finalize()
                o4_block([0, 1, 2], q2c_bc)
                o4_block([3], q2c_bc, dma_fine=True)

        if timing_mode:
            with tc.tile_pool(name="tickp", bufs=1) as tickp:
                tk = tickp.tile([1, 1], f32, tag="tick")
                nc.vector.memset(tk[:], 1.0)
                nc.sync.dma_start(tick_d[:], tk[:])

    nc.compile()
    return nc


def _get_built():
    global _BUILT
    if _BUILT is None:
        _BUILT = _build()
    return _BUILT


def kernel(c, q, w_c, b_c, w_q, b_q, w_cq, b_cq):
    """Full inputs in, full output out. Data-parallel over batch on 8 cores.

    Biases cancel mathematically (softmax shift invariance), so b_* are
    accepted but unused.
    """
    from concourse import bass_utils

    nc = _get_built()
    c = np.ascontiguousarray(np.asarray(c, dtype=np.float32))
    q = np.ascontiguousarray(np.asarray(q, dtype=np.float32))
    wc = np.ascontiguousarray(np.asarray(w_c, dtype=np.float32))
    wq = np.ascontiguousarray(np.asarray(w_q, dtype=np.float32))
    wcq = np.ascontiguousarray(np.asarray(w_cq, dtype=np.float32))

    in_maps = [
        {"c": c[b], "q": q[b], "wc": wc, "wq": wq, "wcq": wcq}
        for b in range(N_CORES)
    ]
    res = bass_utils.run_bass_kernel_spmd(
        nc, in_maps, core_ids=list(range(N_CORES)))
    return np.stack([res.results[b]["out"] for b in range(N_CORES)])
